# revision 16
# baseline (speedup 1.0000x reference)
# Multi-head attention block (B=4, N=2048, DIM=512, H=8, HD=64) on 8 TRN2 cores.
#
# Sharding: core c handles batch b = c//2 and a 4-head group hg = c%2
# (heads 4*hg .. 4*hg+3).  Each core computes a partial output-projection
# part_c = ctx_hg @ Wo[hg] of shape [N, DIM]; the host sums the two partials
# per batch.
#
# Device dataflow (all activations kept transposed, [feature, position]):
#   xT   [512, 2048]  (host-transposed input slice)
#   qT   [256, 2048]  = (Wq_s * scale)^T-free matmul: lhsT=Wq chunk, rhs=xT
#   kT   [256, 2048]  likewise
#   v    [2048, 260]-ish: per-head [n, 65] tiles, col 64 preset to 1.0 so the
#                     PV matmul also produces the softmax denominator
#   simT [j, i] per head = kT_h(jc)^T-stationary @ qT_h  -> exp on ACT ->
#   ctxT [65, i] += v_aug(jc)^T-stationary @ eT(jc)      (row 64 = sum_j e)
#   ctxT normalized by DMA-broadcast reciprocal of row 64, then
#   part [n, 512] = ctxT^T-stationary @ Wo_s.
import os
import numpy as np

B, N, DIM = 4, 2048, 512
HEADS, HD = 8, 64
HG = 2                      # head groups (cores per batch)
MH = HEADS // HG            # heads per core = 4
M = MH * HD                 # per-core hidden slice = 256
P = 128
KC = DIM // P               # 4 contraction chunks for projections
MC = M // P                 # 2 m-chunks
IB = 512                    # i-block (moving free dim)
NB = N // IB                # 4 i-blocks
NCH = N // P                # 16 n/j chunks
SCALE = HD ** -0.5

_CACHE = {}


def _build_nc(reps=1):
    import concourse.bass as bass
    import concourse.tile as tile
    from concourse import bacc, mybir

    F32 = mybir.dt.float32
    F32R = mybir.dt.float32r
    MMDT = F32R if os.environ.get("BASSK_F32R", "1") == "1" else F32

    def mm_cast(ap):
        return ap

    nc = bacc.Bacc(
        "TRN2", target_bir_lowering=False, debug=False, num_devices=8
    )
    xT = nc.dram_tensor("xT", [DIM, N], MMDT, kind="ExternalInput").ap()
    wq = nc.dram_tensor("wq", [DIM, M], MMDT, kind="ExternalInput").ap()
    wk = nc.dram_tensor("wk", [DIM, M], MMDT, kind="ExternalInput").ap()
    wv = nc.dram_tensor("wv", [DIM, M], MMDT, kind="ExternalInput").ap()
    wo = nc.dram_tensor("wo", [M, DIM], MMDT, kind="ExternalInput").ap()
    ones = nc.dram_tensor("ones", [HD], MMDT, kind="ExternalInput").ap()
    out = nc.dram_tensor("out", [N, DIM], F32, kind="ExternalOutput").ap()

    EXP = mybir.ActivationFunctionType.Exp

    with tile.TileContext(nc) as tc:
        from contextlib import ExitStack

        with nc.allow_low_precision(reason="f32r is 12-mantissa-bit rounded fp32"), ExitStack() as ctx:
            persist = ctx.enter_context(tc.tile_pool(name="persist", bufs=1))
            e_pool = ctx.enter_context(tc.tile_pool(name="e", bufs=6))
            r_pool = ctx.enter_context(tc.tile_pool(name="r", bufs=4))
            o_pool = ctx.enter_context(tc.tile_pool(name="o", bufs=4))
            pj = ctx.enter_context(tc.tile_pool(name="pj", bufs=2, space="PSUM"))
            psim = ctx.enter_context(tc.tile_pool(name="psim", bufs=3, space="PSUM"))
            pctx = ctx.enter_context(tc.tile_pool(name="pctx", bufs=2, space="PSUM"))
            pr = ctx.enter_context(tc.tile_pool(name="pr", bufs=1, space="PSUM"))

            xT_sb = persist.tile([P, KC, N], MMDT)
            qT_sb = persist.tile([P, MC, N], MMDT)
            kT_sb = persist.tile([P, MC, N], MMDT)
            ctxT_sb = persist.tile([P, MC, N], MMDT)
            v_sb = persist.tile([P, MH, NCH, HD + 1], MMDT)
            wq_sb = persist.tile([P, KC, M], MMDT)
            wk_sb = persist.tile([P, KC, M], MMDT)
            wv_sb = persist.tile([P, KC, M], MMDT)
            wo_sb = persist.tile([P, MC, DIM], MMDT)
            ones_sb = persist.tile([1, HD], MMDT)
            nc.gpsimd.dma_start(
                ones_sb[:],
                bass.AP(tensor=ones.tensor, offset=ones.offset,
                        ap=[[0, 1], [1, HD]]),
            )

            nc.sync.dma_start(wq_sb[:], wq.rearrange("(c p) m -> p c m", p=P))
            nc.sync.dma_start(wk_sb[:], wk.rearrange("(c p) m -> p c m", p=P))
            nc.sync.dma_start(wv_sb[:], wv.rearrange("(c p) m -> p c m", p=P))
            nc.sync.dma_start(wo_sb[:], wo.rearrange("(c p) m -> p c m", p=P))
            # ones column for the PV denominator trick (cols 0..63 are
            # filled by the V projection below)
            for h in range(MH):
                nc.gpsimd.dma_start(
                    v_sb[:, h, :, HD:HD + 1],
                    bass.AP(tensor=ones.tensor, offset=ones.offset,
                            ap=[[0, P], [0, NCH], [1, 1]]),
                )

            for rep in range(reps):
                _emit_rep(nc, tc, locals())

    nc.compile()
    return nc


def _emit_rep(nc, tc, env):
    from concourse import mybir

    F32 = mybir.dt.float32
    EXP = mybir.ActivationFunctionType.Exp
    (xT, xT_sb, wq_sb, wk_sb, wv_sb, wo_sb, qT_sb, kT_sb, v_sb, ctxT_sb,
     ones_sb, e_pool, r_pool, o_pool, pj, psim, pctx, pr, out, mm_cast,
     MMDT) = (
        env["xT"], env["xT_sb"], env["wq_sb"], env["wk_sb"], env["wv_sb"],
        env["wo_sb"], env["qT_sb"], env["kT_sb"], env["v_sb"], env["ctxT_sb"],
        env["ones_sb"], env["e_pool"], env["r_pool"], env["o_pool"],
        env["pj"], env["psim"], env["pctx"], env["pr"], env["out"],
        env["mm_cast"], env["MMDT"])
    if True:
        if True:
            for c in range(KC):
                nc.sync.dma_start(xT_sb[:, c, :], xT[c * P:(c + 1) * P, :])

            # q/k projections, transposed layout [m, n]
            for wsb, dst in ((wq_sb, qT_sb), (wk_sb, kT_sb)):
                for mc in range(MC):
                    for nb in range(NB):
                        ps = pj.tile([P, IB], F32, tag="pj")
                        for kc in range(KC):
                            nc.tensor.matmul(
                                ps[:],
                                mm_cast(wsb[:, kc, mc * P:(mc + 1) * P]),
                                mm_cast(xT_sb[:, kc, nb * IB:(nb + 1) * IB]),
                                start=(kc == 0),
                                stop=(kc == KC - 1),
                            )
                        nc.vector.tensor_copy(
                            dst[:, mc, nb * IB:(nb + 1) * IB], ps[:]
                        )

            # v projection, natural layout [n, m], scattered into per-head
            # ones-augmented tiles
            for jc in range(NCH):
                ps = pj.tile([P, IB], F32, tag="pj")
                for kc in range(KC):
                    nc.tensor.matmul(
                        ps[:, :M],
                        mm_cast(xT_sb[:, kc, jc * P:(jc + 1) * P]),
                        mm_cast(wv_sb[:, kc, :]),
                        start=(kc == 0),
                        stop=(kc == KC - 1),
                    )
                for h in range(MH):
                    nc.vector.tensor_copy(
                        v_sb[:, h, jc, 0:HD], ps[:, h * HD:(h + 1) * HD]
                    )

            # attention, head-by-head over i-blocks of 512
            for ib in range(NB):
                isl = slice(ib * IB, (ib + 1) * IB)
                for h in range(MH):
                    po = (h % 2) * HD
                    mc = h // 2
                    cps = pctx.tile([HD + 1, IB], F32, tag="pc")
                    for jc in range(NCH):
                        sps = psim.tile([P, IB], F32, tag="ps")
                        nc.tensor.matmul(
                            sps[:],
                            mm_cast(kT_sb[po:po + HD, mc, jc * P:(jc + 1) * P]),
                            mm_cast(qT_sb[po:po + HD, mc, isl]),
                            start=True,
                            stop=True,
                        )
                        et = e_pool.tile([P, IB], MMDT, tag="e")
                        nc.scalar.activation(et[:], sps[:], EXP)
                        nc.tensor.matmul(
                            cps[:],
                            mm_cast(v_sb[:, h, jc, :]),
                            mm_cast(et[:]),
                            start=(jc == 0),
                            stop=(jc == NCH - 1),
                        )
                    rc = r_pool.tile([1, IB], MMDT, tag="rc")
                    nc.vector.reciprocal(rc[:], cps[HD:HD + 1, :])
                    rp = pr.tile([HD, IB], F32, tag="rp")
                    nc.tensor.matmul(
                        rp[:], mm_cast(ones_sb[:]), mm_cast(rc[:]),
                        start=True, stop=True,
                    )
                    rb = r_pool.tile([HD, IB], MMDT, tag="rb")
                    nc.vector.tensor_copy(rb[:], rp[:])
                    nc.vector.tensor_mul(
                        ctxT_sb[po:po + HD, mc, isl], cps[0:HD, :], rb[:]
                    )

            # output projection part = ctx @ Wo_s
            for nck in range(NCH):
                ps = pj.tile([P, IB], F32, tag="pj")
                for mc in range(MC):
                    nc.tensor.matmul(
                        ps[:],
                        mm_cast(ctxT_sb[:, mc, nck * P:(nck + 1) * P]),
                        mm_cast(wo_sb[:, mc, :]),
                        start=(mc == 0),
                        stop=(mc == MC - 1),
                    )
                ot = o_pool.tile([P, IB], F32, tag="ot")
                nc.vector.tensor_copy(ot[:], ps[:])
                nc.sync.dma_start(out[nck * P:(nck + 1) * P, :], ot[:])


def _get_nc(reps=1):
    key = ("nc", reps)
    if key not in _CACHE:
        _CACHE[key] = _build_nc(reps)
    return _CACHE[key]


def make_in_maps(x, Wq, Wkv, Wo):
    x = np.asarray(x, dtype=np.float32)
    Wq = np.asarray(Wq, dtype=np.float32)
    Wkv = np.asarray(Wkv, dtype=np.float32)
    Wo = np.asarray(Wo, dtype=np.float32)
    in_maps = []
    for c in range(8):
        b, hg = divmod(c, HG)
        sl = slice(hg * M, (hg + 1) * M)
        in_maps.append({
            "xT": np.ascontiguousarray(x[b].T),
            "wq": np.ascontiguousarray(Wq[:, sl]) * np.float32(SCALE),
            "wk": np.ascontiguousarray(Wkv[:, :DIM][:, sl]),
            "wv": np.ascontiguousarray(Wkv[:, DIM:][:, sl]),
            "wo": np.ascontiguousarray(Wo[sl, :]),
            "ones": np.ones(HD, dtype=np.float32),
        })
    return in_maps


def gather_out(results):
    out = np.zeros((B, N, DIM), dtype=np.float32)
    for c in range(8):
        out[c // HG] += results[c]["out"]
    return out


def kernel(x, Wq, Wkv, Wo):
    from concourse.bass_utils import run_bass_kernel_spmd

    nc = _get_nc()
    in_maps = make_in_maps(x, Wq, Wkv, Wo)
    res = run_bass_kernel_spmd(nc, in_maps, core_ids=list(range(8)))
    return gather_out(res.results)
